# revision 5
# baseline (speedup 1.0000x reference)
"""Cross-attention Trainium2 kernel (Bass/Tile), data-parallel over batch.

Reference computation per batch element b:
    qp = q[b] @ Wq            [S, O]
    kp = k[b] @ Wk            [S, O]
    vp = k[b] @ Wv            [S, O]
    A  = qp @ kp.T            [S, S]
    W  = softmax(A, axis=-1)  (over key axis)
    C  = W.T @ vp             [S, O]   (contract over the QUERY axis)
    out[b] = concat([q[b], C], axis=-1)

qp and kp feed ONLY the logits, so A = q @ (Wq Wk^T) @ k^T. The weight-
only product M = Wq @ Wk^T is batch-independent and precomputed on the
HOST; the device computes T = M @ k^T (one [D,S] matmul, 2.1G MACs)
instead of both projections (4.3G) — 14% fewer MACs overall:
    T  = M @ k^T              [D, S]   (lhsT = M^T, host-shipped)
    vp = k[b] @ Wv            [S, O]
    A  = q @ T                [S, S]   (lhsT = q^T, host-shipped)
    U  = exp(A - rowmax)      [S, S]  fp16, 1/Z folded into vp rows
    C  = U^T @ vp'            [S, O]  fp16 out, host casts up

Sharding: B=8 batch elements -> 8 NeuronCores, one element per core,
MT/Wv replicated. All device I/O is fp16 (12MB in, 4MB out per core);
every matmul is fp16 with fp32 PSUM accumulation (1 cycle/row — same PE
rate as f32r, half the DMA/SBUF).

All SBUF tiles are allocated ONCE (no per-repeat pool churn), so across
bench repeats the WAR chain alone paces reuse: the next repeat's kT/MT/
Wv DMAs start as soon as phase vp/T of the current repeat stops reading
them and overlap phases A+C; qT is streamed per q-tile during phase A.
Input DMAs ride the SP HWDGE ring, output DMAs the Activation ring, so
C-phase stores never head-of-line-block the next repeat's input loads.

SBUF/partition: kt 32K + mt 16K + wv 16K + t 32K + vp 32K + u 64K
  + qs stream 6K + staging ~6K = ~204K of 208K.
PE work: 786432 cycles = 327.7us @ 2.4GHz; PSUM groups ping-pong
(4+4 or 2+2 banks) so DVE drains overlap PE fill.
"""

import numpy as np

import concourse.bass as bass
import concourse.tile as tile
from concourse import bacc, mybir
from concourse.bass import ts
from concourse.bass_utils import run_bass_kernel_spmd

F32 = mybir.dt.float32
F16 = mybir.dt.float16
AF = mybir.ActivationFunctionType
AX = mybir.AxisListType

P = 128  # SBUF partitions

# Full problem geometry (hardcoded: the harness calls kernel() with these)
B_FULL, S_FULL, D_FULL, O_FULL = 8, 2048, 1024, 1024
N_CORES = 8


def build_nc(S=S_FULL, D=D_FULL, O=O_FULL, repeat=1):
    """Build + compile the per-core Bass module.

    DRAM I/O (all fp16, host pre-laid-out):
      qT  [D, S]  = q[b].T          MT  [D, D]  = (Wq @ Wk^T)^T = Wk @ Wq^T
      kT  [D, S]  = k[b].T          Wv  [D, O]
      C   [S, O]  context (fp16; host casts to f32 and concats with q)

    SBUF tiles are [partition, ...free]; contraction dim always lands on
    partitions, d = chunk*128 + p.
    """
    NB = 512            # moving free-dim block (one PSUM bank of f32)
    DC = D // P         # contraction chunks (d or e)
    QT = S // P         # q partition tiles
    KT = S // P         # key partition tiles
    SB = S // NB        # s blocks
    OB = O // NB        # o blocks
    KB = S // NB        # kk blocks inside one q-tile's logits row

    nc = bacc.Bacc("TRN2", target_bir_lowering=False, debug=False)

    qT = nc.dram_tensor("qT", [D, S], F16, kind="ExternalInput").ap()
    kT = nc.dram_tensor("kT", [D, S], F16, kind="ExternalInput").ap()
    mt = nc.dram_tensor("MT", [D, D], F16, kind="ExternalInput").ap()
    wv = nc.dram_tensor("Wv", [D, O], F16, kind="ExternalInput").ap()
    out = nc.dram_tensor("C", [S, O], F16, kind="ExternalOutput").ap()

    qT_v = qT.rearrange("(c p) s -> p c s", p=P)
    kT_v = kT.rearrange("(c p) s -> p c s", p=P)
    mt_v = mt.rearrange("(c p) d -> p c d", p=P)
    wv_v = wv.rearrange("(c p) o -> p c o", p=P)
    out_v = out.rearrange("(t p) o -> p t o", p=P)

    with tile.TileContext(nc, pool_alloc_mode="queue") as tc:
        with (
            tc.tile_pool(name="ps", bufs=8, space="PSUM") as psum,
            tc.tile_pool(name="stats", bufs=4) as stats,
            tc.tile_pool(name="stage", bufs=4) as stage,
            tc.tile_pool(name="qs", bufs=3) as qs_pool,
            tc.tile_pool(name="kt", bufs=1) as kt_pool,
            tc.tile_pool(name="mt", bufs=1) as mt_pool,
            tc.tile_pool(name="wv", bufs=1) as wv_pool,
            tc.tile_pool(name="t", bufs=1) as t_pool,
            tc.tile_pool(name="vp", bufs=1) as vp_pool,
            tc.tile_pool(name="u", bufs=1) as u_pool,
        ):
            kt_sb = kt_pool.tile([P, DC, S], F16)
            mt_sb = mt_pool.tile([P, DC, D], F16)
            wv_sb = wv_pool.tile([P, DC, O], F16)
            t_sb = t_pool.tile([P, DC, S], F16)
            vp_sb = vp_pool.tile([P, QT, O], F16)
            u_sb = u_pool.tile([P, QT, S], F16)

            for _rep in range(repeat):
                # Input DMAs (SP ring) in need-order: MT first-half cols +
                # kT[sb=0] gate the first matmul group; Wv gates phase vp;
                # qT is streamed later, inside phase A.
                for ec in range(DC):
                    nc.sync.dma_start(
                        out=mt_sb[:, ec, 0:NB], in_=mt_v[:, ec, 0:NB]
                    )
                for ec in range(DC):
                    nc.sync.dma_start(
                        out=kt_sb[:, ec, 0:NB], in_=kT_v[:, ec, 0:NB]
                    )
                for ec in range(DC):
                    nc.sync.dma_start(
                        out=mt_sb[:, ec, NB:D], in_=mt_v[:, ec, NB:D]
                    )
                for sb in range(1, SB):
                    for ec in range(DC):
                        nc.sync.dma_start(
                            out=kt_sb[:, ec, ts(sb, NB)],
                            in_=kT_v[:, ec, ts(sb, NB)],
                        )
                for dc in range(DC):
                    nc.sync.dma_start(out=wv_sb[:, dc, :], in_=wv_v[:, dc, :])

                # ---- Phase T: T = M @ k^T, resident [P, DC, S] ----------
                # 4+4 PSUM ping-pong: group g's 4 banks accumulate over e
                # while group 1-g drains to SBUF.
                for sb in range(SB):
                    for g in range(2):
                        pss = [
                            psum.tile([P, NB], F32, tag="ps", name=f"ps_t{j}")
                            for j in range(4)
                        ]
                        for ec in range(DC):
                            for j in range(4):
                                nc.tensor.matmul(
                                    pss[j],
                                    mt_sb[:, ec, ts(g * 4 + j, P)],
                                    kt_sb[:, ec, ts(sb, NB)],
                                    start=(ec == 0),
                                    stop=(ec == DC - 1),
                                )
                        for j in range(4):
                            nc.vector.tensor_copy(
                                out=t_sb[:, g * 4 + j, ts(sb, NB)], in_=pss[j]
                            )

                # ---- Phase vp: vp = k @ Wv, resident [P, QT, O] ---------
                for st in range(QT):
                    ps2 = [
                        psum.tile([P, NB], F32, tag="ps", name=f"ps_v{ob}")
                        for ob in range(OB)
                    ]
                    for dc in range(DC):
                        for ob in range(OB):
                            nc.tensor.matmul(
                                ps2[ob],
                                kt_sb[:, dc, ts(st, P)],
                                wv_sb[:, dc, ts(ob, NB)],
                                start=(dc == 0),
                                stop=(dc == DC - 1),
                            )
                    for ob in range(OB):
                        nc.vector.tensor_copy(
                            out=vp_sb[:, st, ts(ob, NB)], in_=ps2[ob]
                        )

                # ---- Phase A: logits + softmax, U resident --------------
                # qT streams per q-tile: [P, DC, 128] tiles, 3-deep pool.
                for qt in range(QT):
                    qs = qs_pool.tile([P, DC, P], F16, tag="qs")
                    for dc in range(DC):
                        nc.sync.dma_start(
                            out=qs[:, dc, :], in_=qT_v[:, dc, ts(qt, P)]
                        )
                    a_ps = [
                        psum.tile([P, NB], F32, tag="ps", name=f"ps_a{kb}")
                        for kb in range(KB)
                    ]
                    for dc in range(DC):
                        for kb in range(KB):
                            nc.tensor.matmul(
                                a_ps[kb],
                                qs[:, dc, :],
                                t_sb[:, dc, ts(kb, NB)],
                                start=(dc == 0),
                                stop=(dc == DC - 1),
                            )
                    bmax = stats.tile([P, KB], F32, tag="bmax")
                    for kb in range(KB):
                        nc.vector.reduce_max(
                            out=bmax[:, kb : kb + 1], in_=a_ps[kb], axis=AX.X
                        )
                    negmax = stats.tile([P, 1], F32, tag="negmax")
                    nc.vector.reduce_max(
                        out=negmax, in_=bmax, axis=AX.X, negate=True
                    )
                    zblk = stats.tile([P, KB], F32, tag="zblk")
                    for kb in range(KB):
                        nc.scalar.activation(
                            out=u_sb[:, qt, ts(kb, NB)],
                            in_=a_ps[kb],
                            func=AF.Exp,
                            bias=negmax,
                            scale=1.0,
                            accum_out=zblk[:, kb : kb + 1],
                        )
                    z = stats.tile([P, 1], F32, tag="z")
                    nc.vector.reduce_sum(out=z, in_=zblk, axis=AX.X)
                    rz = stats.tile([P, 1], F32, tag="rz")
                    nc.vector.reciprocal(out=rz, in_=z)
                    # fold 1/Z into vp rows of this q-tile
                    nc.vector.tensor_scalar_mul(
                        vp_sb[:, qt, :], vp_sb[:, qt, :], rz
                    )

                # ---- Phase C: C[kk,o] = sum_q U^T @ vp' -----------------
                for kt in range(KT):
                    ps2 = [
                        psum.tile([P, NB], F32, tag="ps", name=f"ps_c{ob}")
                        for ob in range(OB)
                    ]
                    for qt in range(QT):
                        for ob in range(OB):
                            nc.tensor.matmul(
                                ps2[ob],
                                u_sb[:, qt, ts(kt, P)],
                                vp_sb[:, qt, ts(ob, NB)],
                                start=(qt == 0),
                                stop=(qt == QT - 1),
                            )
                    for ob in range(OB):
                        cst = stage.tile([P, NB], F16, tag="cst")
                        nc.vector.tensor_copy(out=cst, in_=ps2[ob])
                        # output DMA on the Activation HWDGE ring
                        nc.scalar.dma_start(
                            out=out_v[:, kt, ts(ob, NB)], in_=cst
                        )

    nc.compile()
    return nc


_CACHE = {}

# Set TRACE=True (e.g. from a test harness) to capture an NTFF profile;
# LAST_RESULT then holds the BassKernelResults with exec_time_ns.
TRACE = False
LAST_RESULT = None


def _get_nc():
    if "nc" not in _CACHE:
        _CACHE["nc"] = build_nc()
    return _CACHE["nc"]


def prep_in_maps(q, k, Wq, Wk, Wv):
    """Per-core input maps (host-side shard/layout prep), shared with bench."""
    B = q.shape[0]
    # MT = (Wq @ Wk^T)^T = Wk @ Wq^T — batch-independent, done once on host
    mt = (np.asarray(Wk, np.float32) @ np.asarray(Wq, np.float32).T).astype(
        np.float16
    )
    wv = np.ascontiguousarray(Wv, dtype=np.float16)
    in_maps = []
    for b in range(B):
        in_maps.append(
            {
                "qT": np.ascontiguousarray(q[b].T.astype(np.float16)),
                "kT": np.ascontiguousarray(k[b].T.astype(np.float16)),
                "MT": mt,
                "Wv": wv,
            }
        )
    return in_maps


def kernel(q, k, Wq, Wk, Wv):
    """Full-input entry point: q,k [B,S,D] f32; Wq/Wk/Wv [D,O] f32.

    Returns [B, S, D+O] f32 (= concat([q, context], -1) per reference).
    """
    nc = _get_nc()
    B = q.shape[0]
    in_maps = prep_in_maps(q, k, Wq, Wk, Wv)
    global LAST_RESULT
    res = run_bass_kernel_spmd(
        nc, in_maps, core_ids=list(range(N_CORES)), trace=TRACE
    )
    LAST_RESULT = res
    ctx = np.stack(
        [res.results[b]["C"].astype(np.float32) for b in range(B)], axis=0
    )
    return np.concatenate([np.asarray(q, dtype=np.float32), ctx], axis=-1)


# revision 8
# speedup vs baseline: 1.2554x; 1.2554x over previous
"""Cross-attention Trainium2 kernel (Bass/Tile), data-parallel over batch.

Reference computation per batch element b:
    qp = q[b] @ Wq            [S, O]
    kp = k[b] @ Wk            [S, O]
    vp = k[b] @ Wv            [S, O]
    A  = qp @ kp.T            [S, S]
    W  = softmax(A, axis=-1)  (over key axis)
    C  = W.T @ vp             [S, O]   (contract over the QUERY axis)
    out[b] = concat([q[b], C], axis=-1)

qp and kp feed ONLY the logits, so A = q @ (Wq Wk^T) @ k^T. The weight-
only product M = Wq @ Wk^T is batch-independent and precomputed on the
HOST; the device computes T = M @ k^T (one [D,S] matmul, 2.1G MACs)
instead of both projections (4.3G) — 14% fewer MACs overall:
    T  = M @ k^T              [D, S]   (lhsT = M^T, host-shipped)
    vp = k[b] @ Wv            [S, O]
    A  = q @ T                [S, S]   (lhsT = q^T, host-shipped)
    U  = exp(A - rowmax)      [S, S]  fp16, 1/Z folded into vp rows
    C  = U^T @ vp'            [S, O]  fp16 out, host casts up

Sharding: B=8 batch elements -> 8 NeuronCores, one element per core,
MT/Wv replicated. All device I/O is fp16 (12MB in, 4MB out per core);
every matmul is fp16 with fp32 PSUM accumulation (1 cycle/row — same PE
rate as f32r, half the DMA/SBUF).

All SBUF tiles are allocated ONCE (no per-repeat pool churn), so across
bench repeats the WAR chain alone paces reuse: the next repeat's kT/MT/
Wv DMAs start as soon as phase vp/T of the current repeat stops reading
them and overlap phases A+C; qT is streamed per q-tile during phase A.
Input DMAs ride the SP HWDGE ring, output DMAs the Activation ring, so
C-phase stores never head-of-line-block the next repeat's input loads.

SBUF/partition: kt 32K + mt 16K + wv 16K + t 32K + vp 32K + u 64K
  + qs stream 6K + staging ~6K = ~204K of 208K.
PE work: 786432 cycles = 327.7us @ 2.4GHz; PSUM groups ping-pong
(4+4 or 2+2 banks) so DVE drains overlap PE fill.
"""

import numpy as np

import concourse.bass as bass
import concourse.tile as tile
from concourse import bacc, mybir
from concourse.bass import ts
from concourse.bass_utils import run_bass_kernel_spmd

F32 = mybir.dt.float32
F16 = mybir.dt.float16
F8 = mybir.dt.float8e4
DR = mybir.MatmulPerfMode.DoubleRow
AF = mybir.ActivationFunctionType
AX = mybir.AxisListType

P = 128  # SBUF partitions

# Full problem geometry (hardcoded: the harness calls kernel() with these)
B_FULL, S_FULL, D_FULL, O_FULL = 8, 2048, 1024, 1024
N_CORES = 8


# q-tile PAIRS of the phase-C contraction run as fp8e4m3 DoubleRow matmuls
# (2x PE rate). 8 = all 16 q-tiles in fp8 (fastest); 0 = all fp16.
# Accuracy (measured on the harness data): NF8=8 -> rel 1.85e-2,
# NF8=4 -> ~1.3e-2, NF8=0 -> 1.8e-3 (gate: 2e-2).
NF8 = 8


def build_nc(S=S_FULL, D=D_FULL, O=O_FULL, repeat=1, nf8=None):
    """Build + compile the per-core Bass module.

    DRAM I/O (all fp16, host pre-laid-out):
      qT  [D, S]  = q[b].T          MT  [D, D]  = (Wq @ Wk^T)^T = Wk @ Wq^T
      kT  [D, S]  = k[b].T          Wv  [D, O]
      C   [S, O]  context (fp16; host casts to f32 and concats with q)

    SBUF tiles are [partition, ...free]; contraction dim always lands on
    partitions, d = chunk*128 + p.
    """
    if nf8 is None:
        nf8 = NF8
    NB = 512            # moving free-dim block (one PSUM bank of f32)
    DC = D // P         # contraction chunks (d or e)
    QT = S // P         # q partition tiles
    KT = S // P         # key partition tiles
    SB = S // NB        # s blocks
    OB = O // NB        # o blocks
    KB = S // NB        # kk blocks inside one q-tile's logits row

    nc = bacc.Bacc("TRN2", target_bir_lowering=False, debug=False)

    qT = nc.dram_tensor("qT", [D, S], F16, kind="ExternalInput").ap()
    kT = nc.dram_tensor("kT", [D, S], F16, kind="ExternalInput").ap()
    mt = nc.dram_tensor("MT", [D, D], F16, kind="ExternalInput").ap()
    wv = nc.dram_tensor("Wv", [D, O], F16, kind="ExternalInput").ap()
    out = nc.dram_tensor("C", [S, O], F16, kind="ExternalOutput").ap()

    qT_v = qT.rearrange("(c p) s -> p c s", p=P)
    kT_v = kT.rearrange("(c p) s -> p c s", p=P)
    mt_v = mt.rearrange("(c p) d -> p c d", p=P)
    wv_v = wv.rearrange("(c p) o -> p c o", p=P)
    out_v = out.rearrange("(t p) o -> p t o", p=P)

    with tile.TileContext(nc, pool_alloc_mode="queue") as tc:
        with (
            tc.tile_pool(name="ps", bufs=8, space="PSUM") as psum,
            tc.tile_pool(name="stats", bufs=4) as stats,
            tc.tile_pool(name="stage", bufs=4) as stage,
            tc.tile_pool(name="qs", bufs=3) as qs_pool,
            tc.tile_pool(name="kt", bufs=1) as kt_pool,
            tc.tile_pool(name="mt", bufs=1) as mt_pool,
            tc.tile_pool(name="wv", bufs=1) as wv_pool,
            tc.tile_pool(name="t", bufs=1) as t_pool,
            tc.tile_pool(name="vp", bufs=1) as vp_pool,
            tc.tile_pool(name="u", bufs=1) as u_pool,
        ):
            kt_sb = kt_pool.tile([P, DC, S], F16)
            mt_sb = mt_pool.tile([P, DC, D], F16)
            wv_sb = wv_pool.tile([P, DC, O], F16)
            t_sb = t_pool.tile([P, DC, S], F16)
            vp_sb = vp_pool.tile([P, QT, O], F16)
            NQ8 = 2 * nf8  # q-tiles whose U/vp' are stored fp8
            u8_sb = (
                u_pool.tile([P, NQ8, S], F8, name="u8_sb") if NQ8 else None
            )
            v8_sb = (
                u_pool.tile([P, NQ8, O], F8, name="v8_sb") if NQ8 else None
            )
            u_sb = (
                u_pool.tile([P, QT - NQ8, S], F16, name="u_sb")
                if NQ8 < QT
                else None
            )

            for _rep in range(repeat):
                # Input DMAs (SP ring) in need-order: MT first-half cols +
                # kT[sb=0] gate the first matmul group; Wv gates phase vp;
                # qT is streamed later, inside phase A.
                for ec in range(DC):
                    nc.sync.dma_start(
                        out=mt_sb[:, ec, 0:NB], in_=mt_v[:, ec, 0:NB]
                    )
                for ec in range(DC):
                    nc.sync.dma_start(
                        out=kt_sb[:, ec, 0:NB], in_=kT_v[:, ec, 0:NB]
                    )
                for ec in range(DC):
                    nc.sync.dma_start(
                        out=mt_sb[:, ec, NB:D], in_=mt_v[:, ec, NB:D]
                    )
                for sb in range(1, SB):
                    for ec in range(DC):
                        nc.sync.dma_start(
                            out=kt_sb[:, ec, ts(sb, NB)],
                            in_=kT_v[:, ec, ts(sb, NB)],
                        )
                for dc in range(DC):
                    nc.sync.dma_start(out=wv_sb[:, dc, :], in_=wv_v[:, dc, :])

                # ---- Phase T: T = M @ k^T, resident [P, DC, S] ----------
                # 4+4 PSUM ping-pong: group g's 4 banks accumulate over e
                # while group 1-g drains to SBUF.
                for sb in range(SB):
                    for g in range(2):
                        pss = [
                            psum.tile([P, NB], F32, tag="ps", name=f"ps_t{j}")
                            for j in range(4)
                        ]
                        for ec in range(DC):
                            for j in range(4):
                                nc.tensor.matmul(
                                    pss[j],
                                    mt_sb[:, ec, ts(g * 4 + j, P)],
                                    kt_sb[:, ec, ts(sb, NB)],
                                    start=(ec == 0),
                                    stop=(ec == DC - 1),
                                )
                        for j in range(4):
                            nc.vector.tensor_copy(
                                out=t_sb[:, g * 4 + j, ts(sb, NB)], in_=pss[j]
                            )

                # ---- Phase vp: vp = k @ Wv, resident [P, QT, O] ---------
                for st in range(QT):
                    ps2 = [
                        psum.tile([P, NB], F32, tag="ps", name=f"ps_v{ob}")
                        for ob in range(OB)
                    ]
                    for dc in range(DC):
                        for ob in range(OB):
                            nc.tensor.matmul(
                                ps2[ob],
                                kt_sb[:, dc, ts(st, P)],
                                wv_sb[:, dc, ts(ob, NB)],
                                start=(dc == 0),
                                stop=(dc == DC - 1),
                            )
                    for ob in range(OB):
                        nc.vector.tensor_copy(
                            out=vp_sb[:, st, ts(ob, NB)], in_=ps2[ob]
                        )

                # ---- Phase A: logits + softmax, U resident --------------
                # qT streams per q-tile: [P, DC, 128] tiles, 3-deep pool.
                for qt in range(QT):
                    qs = qs_pool.tile([P, DC, P], F16, tag="qs")
                    for dc in range(DC):
                        nc.sync.dma_start(
                            out=qs[:, dc, :], in_=qT_v[:, dc, ts(qt, P)]
                        )
                    a_ps = [
                        psum.tile([P, NB], F32, tag="ps", name=f"ps_a{kb}")
                        for kb in range(KB)
                    ]
                    for dc in range(DC):
                        for kb in range(KB):
                            nc.tensor.matmul(
                                a_ps[kb],
                                qs[:, dc, :],
                                t_sb[:, dc, ts(kb, NB)],
                                start=(dc == 0),
                                stop=(dc == DC - 1),
                            )
                    bmax = stats.tile([P, KB], F32, tag="bmax")
                    for kb in range(KB):
                        nc.vector.reduce_max(
                            out=bmax[:, kb : kb + 1], in_=a_ps[kb], axis=AX.X
                        )
                    negmax = stats.tile([P, 1], F32, tag="negmax")
                    nc.vector.reduce_max(
                        out=negmax, in_=bmax, axis=AX.X, negate=True
                    )
                    zblk = stats.tile([P, KB], F32, tag="zblk")
                    u_dst = (
                        u8_sb[:, qt, :] if qt < NQ8 else u_sb[:, qt - NQ8, :]
                    )
                    for kb in range(KB):
                        nc.scalar.activation(
                            out=u_dst[:, ts(kb, NB)],
                            in_=a_ps[kb],
                            func=AF.Exp,
                            bias=negmax,
                            scale=1.0,
                            accum_out=zblk[:, kb : kb + 1],
                        )
                    z = stats.tile([P, 1], F32, tag="z")
                    nc.vector.reduce_sum(out=z, in_=zblk, axis=AX.X)
                    rz = stats.tile([P, 1], F32, tag="rz")
                    nc.vector.reciprocal(out=rz, in_=z)
                    # fold 1/Z into vp rows of this q-tile (fp8 tiles get
                    # a scaled fp8 copy; vp_sb itself stays unscaled then)
                    if qt < NQ8:
                        nc.vector.tensor_scalar_mul(
                            v8_sb[:, qt, :], vp_sb[:, qt, :], rz
                        )
                    else:
                        nc.vector.tensor_scalar_mul(
                            vp_sb[:, qt, :], vp_sb[:, qt, :], rz
                        )

                # ---- Phase C: C[kk,o] = sum_q U^T @ vp' -----------------
                for kt in range(KT):
                    ps2 = [
                        psum.tile([P, NB], F32, tag="ps", name=f"ps_c{ob}")
                        for ob in range(OB)
                    ]
                    for qp in range(nf8):
                        for ob in range(OB):
                            nc.tensor.matmul(
                                ps2[ob],
                                u8_sb[:, 2 * qp : 2 * qp + 2, ts(kt, P)],
                                v8_sb[:, 2 * qp : 2 * qp + 2, ts(ob, NB)],
                                start=(qp == 0),
                                stop=(qp == nf8 - 1 and NQ8 == QT),
                                perf_mode=DR,
                            )
                    for qt in range(NQ8, QT):
                        for ob in range(OB):
                            nc.tensor.matmul(
                                ps2[ob],
                                u_sb[:, qt - NQ8, ts(kt, P)],
                                vp_sb[:, qt, ts(ob, NB)],
                                start=(nf8 == 0 and qt == NQ8),
                                stop=(qt == QT - 1),
                            )
                    for ob in range(OB):
                        cst = stage.tile([P, NB], F16, tag="cst")
                        nc.vector.tensor_copy(out=cst, in_=ps2[ob])
                        # output DMA on the Activation HWDGE ring
                        nc.scalar.dma_start(
                            out=out_v[:, kt, ts(ob, NB)], in_=cst
                        )

    nc.compile()
    return nc


_CACHE = {}

# Set TRACE=True (e.g. from a test harness) to capture an NTFF profile;
# LAST_RESULT then holds the BassKernelResults with exec_time_ns.
TRACE = False
LAST_RESULT = None


def _get_nc():
    if "nc" not in _CACHE:
        _CACHE["nc"] = build_nc()
    return _CACHE["nc"]


def prep_in_maps(q, k, Wq, Wk, Wv):
    """Per-core input maps (host-side shard/layout prep), shared with bench."""
    B = q.shape[0]
    # MT = (Wq @ Wk^T)^T = Wk @ Wq^T — batch-independent, done once on host
    mt = (np.asarray(Wk, np.float32) @ np.asarray(Wq, np.float32).T).astype(
        np.float16
    )
    wv = np.ascontiguousarray(Wv, dtype=np.float16)
    in_maps = []
    for b in range(B):
        in_maps.append(
            {
                "qT": np.ascontiguousarray(q[b].T.astype(np.float16)),
                "kT": np.ascontiguousarray(k[b].T.astype(np.float16)),
                "MT": mt,
                "Wv": wv,
            }
        )
    return in_maps


def kernel(q, k, Wq, Wk, Wv):
    """Full-input entry point: q,k [B,S,D] f32; Wq/Wk/Wv [D,O] f32.

    Returns [B, S, D+O] f32 (= concat([q, context], -1) per reference).
    """
    nc = _get_nc()
    B = q.shape[0]
    in_maps = prep_in_maps(q, k, Wq, Wk, Wv)
    global LAST_RESULT
    res = run_bass_kernel_spmd(
        nc, in_maps, core_ids=list(range(N_CORES)), trace=TRACE
    )
    LAST_RESULT = res
    ctx = np.stack(
        [res.results[b]["C"].astype(np.float32) for b in range(B)], axis=0
    )
    return np.concatenate([np.asarray(q, dtype=np.float32), ctx], axis=-1)


# revision 9
# speedup vs baseline: 1.4056x; 1.1196x over previous
"""Cross-attention Trainium2 kernel (Bass/Tile), data-parallel over batch.

Reference computation per batch element b:
    qp = q[b] @ Wq            [S, O]
    kp = k[b] @ Wk            [S, O]
    vp = k[b] @ Wv            [S, O]
    A  = qp @ kp.T            [S, S]
    W  = softmax(A, axis=-1)  (over key axis)
    C  = W.T @ vp             [S, O]   (contract over the QUERY axis)
    out[b] = concat([q[b], C], axis=-1)

qp and kp feed ONLY the logits, so A = q @ (Wq Wk^T) @ k^T. The weight-
only product M = Wq @ Wk^T is batch-independent and precomputed on the
HOST; the device computes T = M @ k^T (one [D,S] matmul, 2.1G MACs)
instead of both projections (4.3G) — 14% fewer MACs overall:
    T  = M @ k^T              [D, S]   (lhsT = M^T, host-shipped)
    vp = k[b] @ Wv            [S, O]
    A  = q @ T                [S, S]   (lhsT = q^T, host-shipped)
    U  = exp(A - rowmax)      [S, S]  fp16, 1/Z folded into vp rows
    C  = U^T @ vp'            [S, O]  fp16 out, host casts up

Sharding: B=8 batch elements -> 8 NeuronCores, one element per core,
MT/Wv replicated. All device I/O is fp16 (12MB in, 4MB out per core);
every matmul is fp16 with fp32 PSUM accumulation (1 cycle/row — same PE
rate as f32r, half the DMA/SBUF).

All SBUF tiles are allocated ONCE (no per-repeat pool churn), so across
bench repeats the WAR chain alone paces reuse: the next repeat's kT/MT/
Wv DMAs start as soon as phase vp/T of the current repeat stops reading
them and overlap phases A+C; qT is streamed per q-tile during phase A.
Input DMAs ride the SP HWDGE ring, output DMAs the Activation ring, so
C-phase stores never head-of-line-block the next repeat's input loads.

SBUF/partition: kt 32K + mt 16K + wv 16K + t 32K + vp 32K + u 64K
  + qs stream 6K + staging ~6K = ~204K of 208K.
PE work: 786432 cycles = 327.7us @ 2.4GHz; PSUM groups ping-pong
(4+4 or 2+2 banks) so DVE drains overlap PE fill.
"""

import numpy as np

import concourse.bass as bass
import concourse.tile as tile
from concourse import bacc, mybir
from concourse.bass import ts
from concourse.bass_utils import run_bass_kernel_spmd

F32 = mybir.dt.float32
F16 = mybir.dt.float16
F8 = mybir.dt.float8e4
DR = mybir.MatmulPerfMode.DoubleRow
AF = mybir.ActivationFunctionType
AX = mybir.AxisListType

P = 128  # SBUF partitions

# Full problem geometry (hardcoded: the harness calls kernel() with these)
B_FULL, S_FULL, D_FULL, O_FULL = 8, 2048, 1024, 1024
N_CORES = 8


# q-tile PAIRS of the phase-C contraction run as fp8e4m3 DoubleRow matmuls
# (2x PE rate). 8 = all 16 q-tiles in fp8 (fastest); 0 = all fp16.
# Accuracy (measured on the harness data): NF8=8 -> rel 1.85e-2,
# NF8=4 -> ~1.3e-2, NF8=0 -> 1.8e-3 (gate: 2e-2).
NF8 = 7


def build_nc(S=S_FULL, D=D_FULL, O=O_FULL, repeat=1, nf8=None):
    """Build + compile the per-core Bass module.

    DRAM I/O (all fp16, host pre-laid-out):
      qT  [D, S]  = q[b].T          MT  [D, D]  = (Wq @ Wk^T)^T = Wk @ Wq^T
      kT  [D, S]  = k[b].T          Wv  [D, O]
      C   [S, O]  context (fp16; host casts to f32 and concats with q)

    SBUF tiles are [partition, ...free]; contraction dim always lands on
    partitions, d = chunk*128 + p.
    """
    if nf8 is None:
        nf8 = NF8
    NB = 512            # moving free-dim block (one PSUM bank of f32)
    DC = D // P         # contraction chunks (d or e)
    QT = S // P         # q partition tiles
    KT = S // P         # key partition tiles
    SB = S // NB        # s blocks
    OB = O // NB        # o blocks
    KB = S // NB        # kk blocks inside one q-tile's logits row

    nc = bacc.Bacc("TRN2", target_bir_lowering=False, debug=False)

    qT = nc.dram_tensor("qT", [D, S], F16, kind="ExternalInput").ap()
    kT = nc.dram_tensor("kT", [D, S], F16, kind="ExternalInput").ap()
    mt = nc.dram_tensor("MT", [D, D], F16, kind="ExternalInput").ap()
    wv = nc.dram_tensor("Wv", [D, O], F16, kind="ExternalInput").ap()
    out = nc.dram_tensor("C", [S, O], F16, kind="ExternalOutput").ap()

    qT_v = qT.rearrange("(c p) s -> p c s", p=P)
    kT_v = kT.rearrange("(c p) s -> p c s", p=P)
    mt_v = mt.rearrange("(c p) d -> p c d", p=P)
    wv_v = wv.rearrange("(c p) o -> p c o", p=P)
    out_v = out.rearrange("(t p) o -> p t o", p=P)

    with tile.TileContext(nc, pool_alloc_mode="queue") as tc:
        with (
            tc.tile_pool(name="ps", bufs=8, space="PSUM") as psum,
            tc.tile_pool(name="stats", bufs=4) as stats,
            tc.tile_pool(name="stage", bufs=4) as stage,
            tc.tile_pool(name="qs", bufs=3) as qs_pool,
            tc.tile_pool(name="kt", bufs=1) as kt_pool,
            tc.tile_pool(name="mt", bufs=1) as mt_pool,
            tc.tile_pool(name="wv", bufs=1) as wv_pool,
            tc.tile_pool(name="t", bufs=1) as t_pool,
            tc.tile_pool(name="vp", bufs=1) as vp_pool,
            tc.tile_pool(name="u", bufs=1) as u_pool,
        ):
            kt_sb = kt_pool.tile([P, DC, S], F16)
            mt_sb = mt_pool.tile([P, DC, D], F16)
            wv_sb = wv_pool.tile([P, DC, O], F16)
            t_sb = t_pool.tile([P, DC, S], F16)
            vp_sb = vp_pool.tile([P, QT, O], F16)
            NQ8 = 2 * nf8  # q-tiles whose U/vp' are stored fp8
            u8_sb = (
                u_pool.tile([P, NQ8, S], F8, name="u8_sb") if NQ8 else None
            )
            v8_sb = (
                u_pool.tile([P, NQ8, O], F8, name="v8_sb") if NQ8 else None
            )
            u_sb = (
                u_pool.tile([P, QT - NQ8, S], F16, name="u_sb")
                if NQ8 < QT
                else None
            )

            for _rep in range(repeat):
                # Input DMAs (SP ring) in need-order: MT first-half cols +
                # kT[sb=0] gate the first matmul group; Wv gates phase vp;
                # qT is streamed later, inside phase A.
                for ec in range(DC):
                    nc.sync.dma_start(
                        out=mt_sb[:, ec, 0:NB], in_=mt_v[:, ec, 0:NB]
                    )
                for ec in range(DC):
                    nc.sync.dma_start(
                        out=kt_sb[:, ec, 0:NB], in_=kT_v[:, ec, 0:NB]
                    )
                for ec in range(DC):
                    nc.sync.dma_start(
                        out=mt_sb[:, ec, NB:D], in_=mt_v[:, ec, NB:D]
                    )
                for sb in range(1, SB):
                    for ec in range(DC):
                        nc.sync.dma_start(
                            out=kt_sb[:, ec, ts(sb, NB)],
                            in_=kT_v[:, ec, ts(sb, NB)],
                        )
                for dc in range(DC):
                    nc.sync.dma_start(out=wv_sb[:, dc, :], in_=wv_v[:, dc, :])

                # ---- Phase T: T = M @ k^T, resident [P, DC, S] ----------
                # 4+4 PSUM ping-pong: group g's 4 banks accumulate over e
                # while group 1-g drains to SBUF.
                for sb in range(SB):
                    for g in range(2):
                        pss = [
                            psum.tile([P, NB], F32, tag="ps", name=f"ps_t{j}")
                            for j in range(4)
                        ]
                        for ec in range(DC):
                            for j in range(4):
                                nc.tensor.matmul(
                                    pss[j],
                                    mt_sb[:, ec, ts(g * 4 + j, P)],
                                    kt_sb[:, ec, ts(sb, NB)],
                                    start=(ec == 0),
                                    stop=(ec == DC - 1),
                                )
                        for j in range(4):
                            nc.vector.tensor_copy(
                                out=t_sb[:, g * 4 + j, ts(sb, NB)], in_=pss[j]
                            )

                # ---- Phase vp: vp = k @ Wv, resident [P, QT, O] ---------
                for st in range(QT):
                    ps2 = [
                        psum.tile([P, NB], F32, tag="ps", name=f"ps_v{ob}")
                        for ob in range(OB)
                    ]
                    for dc in range(DC):
                        for ob in range(OB):
                            nc.tensor.matmul(
                                ps2[ob],
                                kt_sb[:, dc, ts(st, P)],
                                wv_sb[:, dc, ts(ob, NB)],
                                start=(dc == 0),
                                stop=(dc == DC - 1),
                            )
                    for ob in range(OB):
                        nc.vector.tensor_copy(
                            out=vp_sb[:, st, ts(ob, NB)], in_=ps2[ob]
                        )

                # ---- Phase A: logits + softmax, U resident --------------
                # qT streams per q-tile: [P, DC, 128] tiles, 3-deep pool.
                for qt in range(QT):
                    qs = qs_pool.tile([P, DC, P], F16, tag="qs")
                    for dc in range(DC):
                        nc.sync.dma_start(
                            out=qs[:, dc, :], in_=qT_v[:, dc, ts(qt, P)]
                        )
                    a_ps = [
                        psum.tile([P, NB], F32, tag="ps", name=f"ps_a{kb}")
                        for kb in range(KB)
                    ]
                    for dc in range(DC):
                        for kb in range(KB):
                            nc.tensor.matmul(
                                a_ps[kb],
                                qs[:, dc, :],
                                t_sb[:, dc, ts(kb, NB)],
                                start=(dc == 0),
                                stop=(dc == DC - 1),
                            )
                    bmax = stats.tile([P, KB], F32, tag="bmax")
                    for kb in range(KB):
                        nc.vector.reduce_max(
                            out=bmax[:, kb : kb + 1], in_=a_ps[kb], axis=AX.X
                        )
                    negmax = stats.tile([P, 1], F32, tag="negmax")
                    nc.vector.reduce_max(
                        out=negmax, in_=bmax, axis=AX.X, negate=True
                    )
                    zblk = stats.tile([P, KB], F32, tag="zblk")
                    u_dst = (
                        u8_sb[:, qt, :] if qt < NQ8 else u_sb[:, qt - NQ8, :]
                    )
                    for kb in range(KB):
                        nc.scalar.activation(
                            out=u_dst[:, ts(kb, NB)],
                            in_=a_ps[kb],
                            func=AF.Exp,
                            bias=negmax,
                            scale=1.0,
                            accum_out=zblk[:, kb : kb + 1],
                        )
                    z = stats.tile([P, 1], F32, tag="z")
                    nc.vector.reduce_sum(out=z, in_=zblk, axis=AX.X)
                    rz = stats.tile([P, 1], F32, tag="rz")
                    nc.vector.reciprocal(out=rz, in_=z)
                    # fold 1/Z into vp rows of this q-tile (fp8 tiles get
                    # a scaled fp8 copy; vp_sb itself stays unscaled then)
                    if qt < NQ8:
                        nc.vector.tensor_scalar_mul(
                            v8_sb[:, qt, :], vp_sb[:, qt, :], rz
                        )
                    else:
                        nc.vector.tensor_scalar_mul(
                            vp_sb[:, qt, :], vp_sb[:, qt, :], rz
                        )

                # ---- Phase C: C[kk,o] = sum_q U^T @ vp' -----------------
                for kt in range(KT):
                    ps2 = [
                        psum.tile([P, NB], F32, tag="ps", name=f"ps_c{ob}")
                        for ob in range(OB)
                    ]
                    for qp in range(nf8):
                        for ob in range(OB):
                            nc.tensor.matmul(
                                ps2[ob],
                                u8_sb[:, 2 * qp : 2 * qp + 2, ts(kt, P)],
                                v8_sb[:, 2 * qp : 2 * qp + 2, ts(ob, NB)],
                                start=(qp == 0),
                                stop=(qp == nf8 - 1 and NQ8 == QT),
                                perf_mode=DR,
                            )
                    for qt in range(NQ8, QT):
                        for ob in range(OB):
                            nc.tensor.matmul(
                                ps2[ob],
                                u_sb[:, qt - NQ8, ts(kt, P)],
                                vp_sb[:, qt, ts(ob, NB)],
                                start=(nf8 == 0 and qt == NQ8),
                                stop=(qt == QT - 1),
                            )
                    for ob in range(OB):
                        cst = stage.tile([P, NB], F16, tag="cst")
                        nc.vector.tensor_copy(out=cst, in_=ps2[ob])
                        # output DMA on the Activation HWDGE ring
                        nc.scalar.dma_start(
                            out=out_v[:, kt, ts(ob, NB)], in_=cst
                        )

    nc.compile()
    return nc


_CACHE = {}

# Set TRACE=True (e.g. from a test harness) to capture an NTFF profile;
# LAST_RESULT then holds the BassKernelResults with exec_time_ns.
TRACE = False
LAST_RESULT = None


def _get_nc():
    if "nc" not in _CACHE:
        _CACHE["nc"] = build_nc()
    return _CACHE["nc"]


def prep_in_maps(q, k, Wq, Wk, Wv):
    """Per-core input maps (host-side shard/layout prep), shared with bench."""
    B = q.shape[0]
    # MT = (Wq @ Wk^T)^T = Wk @ Wq^T — batch-independent, done once on host
    mt = (np.asarray(Wk, np.float32) @ np.asarray(Wq, np.float32).T).astype(
        np.float16
    )
    wv = np.ascontiguousarray(Wv, dtype=np.float16)
    in_maps = []
    for b in range(B):
        in_maps.append(
            {
                "qT": np.ascontiguousarray(q[b].T.astype(np.float16)),
                "kT": np.ascontiguousarray(k[b].T.astype(np.float16)),
                "MT": mt,
                "Wv": wv,
            }
        )
    return in_maps


def kernel(q, k, Wq, Wk, Wv):
    """Full-input entry point: q,k [B,S,D] f32; Wq/Wk/Wv [D,O] f32.

    Returns [B, S, D+O] f32 (= concat([q, context], -1) per reference).
    """
    nc = _get_nc()
    B = q.shape[0]
    in_maps = prep_in_maps(q, k, Wq, Wk, Wv)
    global LAST_RESULT
    res = run_bass_kernel_spmd(
        nc, in_maps, core_ids=list(range(N_CORES)), trace=TRACE
    )
    LAST_RESULT = res
    ctx = np.stack(
        [res.results[b]["C"].astype(np.float32) for b in range(B)], axis=0
    )
    return np.concatenate([np.asarray(q, dtype=np.float32), ctx], axis=-1)


# revision 10
# speedup vs baseline: 1.4301x; 1.0175x over previous
"""Cross-attention Trainium2 kernel (Bass/Tile), data-parallel over batch.

Reference computation per batch element b:
    qp = q[b] @ Wq            [S, O]
    kp = k[b] @ Wk            [S, O]
    vp = k[b] @ Wv            [S, O]
    A  = qp @ kp.T            [S, S]
    W  = softmax(A, axis=-1)  (over key axis)
    C  = W.T @ vp             [S, O]   (contract over the QUERY axis)
    out[b] = concat([q[b], C], axis=-1)

qp and kp feed ONLY the logits, so A = q @ (Wq Wk^T) @ k^T. The weight-
only product M = Wq @ Wk^T is batch-independent and precomputed on the
HOST; the device computes T = M @ k^T (one [D,S] matmul, 2.1G MACs)
instead of both projections (4.3G) — 14% fewer MACs overall:
    T  = M @ k^T              [D, S]   (lhsT = M^T, host-shipped)
    vp = k[b] @ Wv            [S, O]
    A  = q @ T                [S, S]   (lhsT = q^T, host-shipped)
    U  = exp(A - rowmax)      [S, S]  fp16/fp8, 1/Z folded into vp rows
    C  = U^T @ vp'            [S, O]  fp16 out, host casts up

Phase C runs 14 of its 16 q-tile contraction steps as fp8e4m3 DoubleRow
matmuls (2x PE rate; NF8 below): measured rel err 1.73e-2 vs the 2e-2
gate (fp16-only would be 1.8e-3; full fp8 1.85e-2).

Sharding: B=8 batch elements -> 8 NeuronCores, one element per core,
MT/Wv replicated. All device I/O is fp16 (12MB in, 4MB out per core);
every matmul is fp16 with fp32 PSUM accumulation (1 cycle/row — same PE
rate as f32r, half the DMA/SBUF).

All SBUF tiles are allocated ONCE (no per-repeat pool churn), so across
bench repeats the WAR chain alone paces reuse: the next repeat's kT/MT/
Wv DMAs start as soon as phase vp/T of the current repeat stops reading
them and overlap phases A+C; qT is streamed per q-tile during phase A.
Input DMAs ride the SP HWDGE ring, output DMAs the Activation ring, so
C-phase stores never head-of-line-block the next repeat's input loads.

SBUF/partition: kt 32K + mt 16K + wv 16K + t 32K + vp 32K
  + u8/v8/u16 ~50K + qs stream 6K + staging ~6K = ~190K of 208K.
PE work: ~671k cycles (T 131k + vp 131k + A 262k + C 147k); PSUM groups
ping-pong (4+4 or 2+2 banks) so DVE drains overlap PE fill.
"""

import numpy as np

import concourse.bass as bass
import concourse.tile as tile
from concourse import bacc, mybir
from concourse.bass import ts
from concourse.bass_utils import run_bass_kernel_spmd

F32 = mybir.dt.float32
F16 = mybir.dt.float16
F8 = mybir.dt.float8e4
DR = mybir.MatmulPerfMode.DoubleRow
AF = mybir.ActivationFunctionType
AX = mybir.AxisListType

P = 128  # SBUF partitions

# Full problem geometry (hardcoded: the harness calls kernel() with these)
B_FULL, S_FULL, D_FULL, O_FULL = 8, 2048, 1024, 1024
N_CORES = 8


# q-tile PAIRS of the phase-C contraction run as fp8e4m3 DoubleRow matmuls
# (2x PE rate). 8 = all 16 q-tiles in fp8 (fastest); 0 = all fp16.
# Accuracy (measured on the harness data, deterministic): NF8=8 -> rel
# 1.846e-2, NF8=7 -> 1.728e-2, NF8=4 -> 1.31e-2, NF8=0 -> 1.80e-3
# (gate: 2e-2). NF8=7 trades ~3% speed for double the gate margin.
NF8 = 7


def build_nc(S=S_FULL, D=D_FULL, O=O_FULL, repeat=1, nf8=None):
    """Build + compile the per-core Bass module.

    DRAM I/O (all fp16, host pre-laid-out):
      qT  [D, S]  = q[b].T          MT  [D, D]  = (Wq @ Wk^T)^T = Wk @ Wq^T
      kT  [D, S]  = k[b].T          Wv  [D, O]
      C   [S, O]  context (fp16; host casts to f32 and concats with q)

    SBUF tiles are [partition, ...free]; contraction dim always lands on
    partitions, d = chunk*128 + p.
    """
    if nf8 is None:
        nf8 = NF8
    NB = 512            # moving free-dim block (one PSUM bank of f32)
    DC = D // P         # contraction chunks (d or e)
    QT = S // P         # q partition tiles
    KT = S // P         # key partition tiles
    SB = S // NB        # s blocks
    OB = O // NB        # o blocks
    KB = S // NB        # kk blocks inside one q-tile's logits row

    nc = bacc.Bacc("TRN2", target_bir_lowering=False, debug=False)

    qT = nc.dram_tensor("qT", [D, S], F16, kind="ExternalInput").ap()
    kT = nc.dram_tensor("kT", [D, S], F16, kind="ExternalInput").ap()
    mt = nc.dram_tensor("MT", [D, D], F16, kind="ExternalInput").ap()
    wv = nc.dram_tensor("Wv", [D, O], F16, kind="ExternalInput").ap()
    out = nc.dram_tensor("C", [S, O], F16, kind="ExternalOutput").ap()

    qT_v = qT.rearrange("(c p) s -> p c s", p=P)
    kT_v = kT.rearrange("(c p) s -> p c s", p=P)
    mt_v = mt.rearrange("(c p) d -> p c d", p=P)
    wv_v = wv.rearrange("(c p) o -> p c o", p=P)
    out_v = out.rearrange("(t p) o -> p t o", p=P)

    with tile.TileContext(nc, pool_alloc_mode="queue") as tc:
        with (
            tc.tile_pool(name="ps", bufs=8, space="PSUM") as psum,
            tc.tile_pool(name="stats", bufs=4) as stats,
            tc.tile_pool(name="stage", bufs=4) as stage,
            tc.tile_pool(name="qs", bufs=3) as qs_pool,
            tc.tile_pool(name="kt", bufs=1) as kt_pool,
            tc.tile_pool(name="mt", bufs=1) as mt_pool,
            tc.tile_pool(name="wv", bufs=1) as wv_pool,
            tc.tile_pool(name="t", bufs=1) as t_pool,
            tc.tile_pool(name="vp", bufs=1) as vp_pool,
            tc.tile_pool(name="u", bufs=1) as u_pool,
        ):
            kt_sb = kt_pool.tile([P, DC, S], F16)
            mt_sb = mt_pool.tile([P, DC, D], F16)
            wv_sb = wv_pool.tile([P, DC, O], F16)
            t_sb = t_pool.tile([P, DC, S], F16)
            vp_sb = vp_pool.tile([P, QT, O], F16)
            NQ8 = 2 * nf8  # q-tiles whose U/vp' are stored fp8
            u8_sb = (
                u_pool.tile([P, NQ8, S], F8, name="u8_sb") if NQ8 else None
            )
            v8_sb = (
                u_pool.tile([P, NQ8, O], F8, name="v8_sb") if NQ8 else None
            )
            u_sb = (
                u_pool.tile([P, QT - NQ8, S], F16, name="u_sb")
                if NQ8 < QT
                else None
            )

            for _rep in range(repeat):
                # Input DMAs (SP ring) in need-order: MT first-half cols +
                # kT[sb=0] gate the first matmul group; Wv gates phase vp;
                # qT is streamed later, inside phase A.
                for ec in range(DC):
                    nc.sync.dma_start(
                        out=mt_sb[:, ec, 0:NB], in_=mt_v[:, ec, 0:NB]
                    )
                for ec in range(DC):
                    nc.sync.dma_start(
                        out=kt_sb[:, ec, 0:NB], in_=kT_v[:, ec, 0:NB]
                    )
                for ec in range(DC):
                    nc.sync.dma_start(
                        out=mt_sb[:, ec, NB:D], in_=mt_v[:, ec, NB:D]
                    )
                for sb in range(1, SB):
                    for ec in range(DC):
                        nc.sync.dma_start(
                            out=kt_sb[:, ec, ts(sb, NB)],
                            in_=kT_v[:, ec, ts(sb, NB)],
                        )
                for dc in range(DC):
                    nc.sync.dma_start(out=wv_sb[:, dc, :], in_=wv_v[:, dc, :])

                # ---- Phase T: T = M @ k^T, resident [P, DC, S] ----------
                # 4+4 PSUM ping-pong: group g's 4 banks accumulate over e
                # while group 1-g drains to SBUF.
                for sb in range(SB):
                    for g in range(2):
                        pss = [
                            psum.tile([P, NB], F32, tag="ps", name=f"ps_t{j}")
                            for j in range(4)
                        ]
                        for ec in range(DC):
                            for j in range(4):
                                nc.tensor.matmul(
                                    pss[j],
                                    mt_sb[:, ec, ts(g * 4 + j, P)],
                                    kt_sb[:, ec, ts(sb, NB)],
                                    start=(ec == 0),
                                    stop=(ec == DC - 1),
                                )
                        for j in range(4):
                            nc.vector.tensor_copy(
                                out=t_sb[:, g * 4 + j, ts(sb, NB)], in_=pss[j]
                            )

                # ---- Phase vp: vp = k @ Wv, resident [P, QT, O] ---------
                for st in range(QT):
                    ps2 = [
                        psum.tile([P, NB], F32, tag="ps", name=f"ps_v{ob}")
                        for ob in range(OB)
                    ]
                    for dc in range(DC):
                        for ob in range(OB):
                            nc.tensor.matmul(
                                ps2[ob],
                                kt_sb[:, dc, ts(st, P)],
                                wv_sb[:, dc, ts(ob, NB)],
                                start=(dc == 0),
                                stop=(dc == DC - 1),
                            )
                    for ob in range(OB):
                        nc.vector.tensor_copy(
                            out=vp_sb[:, st, ts(ob, NB)], in_=ps2[ob]
                        )

                # ---- Phase A: logits + softmax, U resident --------------
                # qT streams per q-tile: [P, DC, 128] tiles, 3-deep pool.
                for qt in range(QT):
                    qs = qs_pool.tile([P, DC, P], F16, tag="qs")
                    for dc in range(DC):
                        nc.sync.dma_start(
                            out=qs[:, dc, :], in_=qT_v[:, dc, ts(qt, P)]
                        )
                    a_ps = [
                        psum.tile([P, NB], F32, tag="ps", name=f"ps_a{kb}")
                        for kb in range(KB)
                    ]
                    for dc in range(DC):
                        for kb in range(KB):
                            nc.tensor.matmul(
                                a_ps[kb],
                                qs[:, dc, :],
                                t_sb[:, dc, ts(kb, NB)],
                                start=(dc == 0),
                                stop=(dc == DC - 1),
                            )
                    bmax = stats.tile([P, KB], F32, tag="bmax")
                    for kb in range(KB):
                        nc.vector.reduce_max(
                            out=bmax[:, kb : kb + 1], in_=a_ps[kb], axis=AX.X
                        )
                    negmax = stats.tile([P, 1], F32, tag="negmax")
                    nc.vector.reduce_max(
                        out=negmax, in_=bmax, axis=AX.X, negate=True
                    )
                    zblk = stats.tile([P, KB], F32, tag="zblk")
                    u_dst = (
                        u8_sb[:, qt, :] if qt < NQ8 else u_sb[:, qt - NQ8, :]
                    )
                    for kb in range(KB):
                        nc.scalar.activation(
                            out=u_dst[:, ts(kb, NB)],
                            in_=a_ps[kb],
                            func=AF.Exp,
                            bias=negmax,
                            scale=1.0,
                            accum_out=zblk[:, kb : kb + 1],
                        )
                    z = stats.tile([P, 1], F32, tag="z")
                    nc.vector.reduce_sum(out=z, in_=zblk, axis=AX.X)
                    rz = stats.tile([P, 1], F32, tag="rz")
                    nc.vector.reciprocal(out=rz, in_=z)
                    # fold 1/Z into vp rows of this q-tile (fp8 tiles get
                    # a scaled fp8 copy; vp_sb itself stays unscaled then)
                    if qt < NQ8:
                        nc.vector.tensor_scalar_mul(
                            v8_sb[:, qt, :], vp_sb[:, qt, :], rz
                        )
                    else:
                        nc.vector.tensor_scalar_mul(
                            vp_sb[:, qt, :], vp_sb[:, qt, :], rz
                        )

                # ---- Phase C: C[kk,o] = sum_q U^T @ vp' -----------------
                for kt in range(KT):
                    ps2 = [
                        psum.tile([P, NB], F32, tag="ps", name=f"ps_c{ob}")
                        for ob in range(OB)
                    ]
                    for qp in range(nf8):
                        for ob in range(OB):
                            nc.tensor.matmul(
                                ps2[ob],
                                u8_sb[:, 2 * qp : 2 * qp + 2, ts(kt, P)],
                                v8_sb[:, 2 * qp : 2 * qp + 2, ts(ob, NB)],
                                start=(qp == 0),
                                stop=(qp == nf8 - 1 and NQ8 == QT),
                                perf_mode=DR,
                            )
                    for qt in range(NQ8, QT):
                        for ob in range(OB):
                            nc.tensor.matmul(
                                ps2[ob],
                                u_sb[:, qt - NQ8, ts(kt, P)],
                                vp_sb[:, qt, ts(ob, NB)],
                                start=(nf8 == 0 and qt == NQ8),
                                stop=(qt == QT - 1),
                            )
                    for ob in range(OB):
                        cst = stage.tile([P, NB], F16, tag="cst")
                        nc.vector.tensor_copy(out=cst, in_=ps2[ob])
                        # output DMA on the Activation HWDGE ring
                        nc.scalar.dma_start(
                            out=out_v[:, kt, ts(ob, NB)], in_=cst
                        )

    nc.compile()
    return nc


_CACHE = {}

# Set TRACE=True (e.g. from a test harness) to capture an NTFF profile;
# LAST_RESULT then holds the BassKernelResults with exec_time_ns.
TRACE = False
LAST_RESULT = None


def _get_nc():
    if "nc" not in _CACHE:
        _CACHE["nc"] = build_nc()
    return _CACHE["nc"]


def prep_in_maps(q, k, Wq, Wk, Wv):
    """Per-core input maps (host-side shard/layout prep), shared with bench."""
    B = q.shape[0]
    # MT = (Wq @ Wk^T)^T = Wk @ Wq^T — batch-independent, done once on host
    mt = (np.asarray(Wk, np.float32) @ np.asarray(Wq, np.float32).T).astype(
        np.float16
    )
    wv = np.ascontiguousarray(Wv, dtype=np.float16)
    in_maps = []
    for b in range(B):
        in_maps.append(
            {
                "qT": np.ascontiguousarray(q[b].T.astype(np.float16)),
                "kT": np.ascontiguousarray(k[b].T.astype(np.float16)),
                "MT": mt,
                "Wv": wv,
            }
        )
    return in_maps


def kernel(q, k, Wq, Wk, Wv):
    """Full-input entry point: q,k [B,S,D] f32; Wq/Wk/Wv [D,O] f32.

    Returns [B, S, D+O] f32 (= concat([q, context], -1) per reference).
    """
    nc = _get_nc()
    B = q.shape[0]
    in_maps = prep_in_maps(q, k, Wq, Wk, Wv)
    global LAST_RESULT
    res = run_bass_kernel_spmd(
        nc, in_maps, core_ids=list(range(N_CORES)), trace=TRACE
    )
    LAST_RESULT = res
    ctx = np.stack(
        [res.results[b]["C"].astype(np.float32) for b in range(B)], axis=0
    )
    return np.concatenate([np.asarray(q, dtype=np.float32), ctx], axis=-1)
